# revision 31
# baseline (speedup 1.0000x reference)
"""BoundaryAttentionModule Trainium2 kernel — moment-expansion algorithm.

Shapes (hardcoded): b=4, c=256, h=w=64 (HW=4096), boundary 128x128,
mid=64, out_ch=256. 8 cores: core = (batch bi = core//2, key-half kh = core%2).

Math: keys K[:,k] = W2 @ relu(kw1f * t_k + beta) depend on the SCALAR
boundary value t_k, so within each linear region S of the 64-breakpoint
piecewise map, E^T[k,j] = t_k * A_S[j] + B_S[j] where
  A_S[j] = sum_{i in S} tau*kw1f_i G[i,j],  B_S[j] = sum_{i in S} beta_i G[i,j]
(G = (W2^T Q_w) @ u, tau = max|t| folded into A). Then exactly
  U[k,j] = exp(B_S[j]) * sum_n ((t_k/tau)^n) * (A_S[j])^n / n!
so with moments Mo[(n,S),c] = sum_{k in S} (t_k/tau)^n v[k,c]/s_k and
w[(n,S),j] = exp(B_S[j]) A_S[j]^n/n!, the attention output collapses to
  P[c,j] = sum_{(n,S)} Mo[(n,S),c] w[(n,S),j].
Softmax denominators s[k] = sum_n (t_k/tau)^n sigma[n,S(k)] with
sigma = row-sums of w (free via accum_out). Truncation N=3 is exact to
well below the bf16 noise floor (final rel err ~1.4e-5; |tau*A| <~ 0.7).
Regions padded/merged to RP=32, so (n,S) = 4*32 = 128 = one partition tile.

Device per core: G2 = M@u, A/B = CAB@G2 (duo across alternating halves),
3-step w-chain on DVE, vT = u_k^T@vw^T (fills PE under the chain),
s = PWT@sigma, PW*(1/s), moment matmul, P matmul. Host: regions,
(t/tau)^n powers, j-roll per key-half, final gamma*P+u.
"""

import numpy as np

B, C, HW = 4, 256, 4096
KH = HW // 2          # 2048 keys per core
NKT = KH // 128       # 16 key tiles
MID = 64
RP = 32               # padded region count
NC = 4                # Taylor orders 0..3
COLS = RP * NC        # 128 = one partition tile

TRACE = False
TRACE_CORES = None
LAST_RESULTS = None

_BUILT = None


def _build():
    import concourse.bass as bass
    import concourse.tile as tile
    from concourse import bacc, mybir

    f32 = mybir.dt.float32
    bf16 = mybir.dt.bfloat16
    AF = mybir.ActivationFunctionType
    AX = mybir.AxisListType
    ALU = mybir.AluOpType

    nc = bacc.Bacc(
        "TRN2",
        target_bir_lowering=False,
        debug=False,
        enable_asserts=False,
        num_devices=8,
    )

    u_in = nc.dram_tensor("u_in", [C, HW], bf16, kind="ExternalInput").ap()
    mt_in = nc.dram_tensor("mt_in", [C, 2 * MID], bf16, kind="ExternalInput").ap()
    # cols 0:128 = 4 replicas of tau*CA^T (A lands in all four 32-row
    # partition groups), cols 128:160 = CB^T; rows duplicated for duo use.
    cab_in = nc.dram_tensor("cab_in", [2 * MID, 5 * RP], bf16, kind="ExternalInput").ap()
    vwt_in = nc.dram_tensor("vwt_in", [C, C], bf16, kind="ExternalInput").ap()
    pw_in = nc.dram_tensor("pw_in", [128, NKT * COLS], bf16, kind="ExternalInput").ap()
    pwt_in = nc.dram_tensor("pwt_in", [COLS, KH], bf16, kind="ExternalInput").ap()
    p_out = nc.dram_tensor("p_out", [C, HW], bf16, kind="ExternalOutput").ap()

    NJC = 8
    JW = HW // NJC        # 512-wide j chunks

    with tile.TileContext(nc) as tc:
        with (
            tc.tile_pool(name="sb", bufs=1) as sb,
            tc.tile_pool(name="ost", bufs=2) as osp,
            tc.tile_pool(name="big", bufs=2, space="PSUM") as bigp,
            tc.tile_pool(name="ab", bufs=2, space="PSUM") as abp,
            tc.tile_pool(name="pin", bufs=1, space="PSUM") as pinp,
        ):
            # ---- input DMAs ----
            mt = sb.tile([128, 2 * MID], bf16, tag="mt", name="mt")
            nc.sync.dma_start(mt[:], mt_in[0:128, :])
            mt1 = sb.tile([128, 2 * MID], bf16, tag="mt1", name="mt1")
            nc.sync.dma_start(mt1[:], mt_in[128:256, :])
            cab = sb.tile([2 * MID, 5 * RP], bf16, tag="cab", name="cab")
            nc.sync.dma_start(cab[:], cab_in[:, :])
            u0 = sb.tile([128, HW], bf16, tag="u0", name="u0")
            u1 = sb.tile([128, HW], bf16, tag="u1", name="u1")
            for jc in range(4):
                jo = jc * 1024
                nc.sync.dma_start(u0[:, jo : jo + 1024], u_in[0:128, jo : jo + 1024])
                nc.gpsimd.dma_start(u1[:, jo : jo + 1024], u_in[128:256, jo : jo + 1024])
            vwt0 = sb.tile([128, C], bf16, tag="vwt0", name="vwt0")
            nc.gpsimd.dma_start(vwt0[:], vwt_in[0:128, :])
            vwt1 = sb.tile([128, C], bf16, tag="vwt1", name="vwt1")
            nc.gpsimd.dma_start(vwt1[:], vwt_in[128:256, :])
            pwsb = sb.tile([128, NKT * COLS], bf16, tag="pwsb", name="pwsb")
            nc.gpsimd.dma_start(pwsb[:], pw_in[:, :])
            pwt = sb.tile([COLS, KH], bf16, tag="pwt", name="pwt")
            nc.gpsimd.dma_start(pwt[:], pwt_in[:, :])

            # ---- SBUF working tiles ----
            G2 = sb.tile([128, HW], bf16, tag="G2", name="G2")
            # tau*A replicated into all four 32-row partition groups so the
            # chain's two SBUF inputs always share a base partition.
            AsclR = sb.tile([128, HW], bf16, tag="AsclR", name="AsclR")
            W0 = sb.tile([128, HW], bf16, tag="W0", name="W0")   # n=0..3
            sacc0 = sb.tile([128, NJC], f32, tag="sacc0", name="sacc0")
            nc.gpsimd.memset(sacc0[:], 0.0)
            sig0 = sb.tile([128, 1], f32, tag="sig0", name="sig0")
            sigb0 = sb.tile([128, 1], bf16, tag="sigb0", name="sigb0")
            rinv = sb.tile([128, NKT], f32, tag="rinv", name="rinv")
            vtb = sb.tile([128, NKT * C], bf16, tag="vtb", name="vtb")
            pws = sb.tile([128, NKT * COLS], bf16, tag="pws", name="pws")
            mo0 = sb.tile([128, C], bf16, tag="mo0", name="mo0")
            kwsrc = sb.tile([32, 8], bf16, tag="kwsrc", name="kwsrc")
            scr = sb.tile([128, 512], bf16, tag="scr", name="scr")
            nc.vector.memset(scr[:], 0.0)   # DVE is idle at t=0

            spin = pinp.tile([128, 512], f32, tag="spin", name="spin")
            s_ps = spin[:, 0:NKT]                  # s accumulators
            mo_ps0 = spin[:, 256 : 256 + C]        # Mo accumulation

            # ---- per 512 j-chunk: G2 matmul+copy, A/B matmuls, expB, AsclR ----
            def g2_chunk(jc):
                jo = jc * JW
                pg = bigp.tile([128, JW], f32, tag="big", name=f"pg{jc}")
                nc.tensor.matmul(
                    pg[:], mt[:, :], u0[:, jo : jo + JW], start=True, stop=False
                )
                nc.tensor.matmul(
                    pg[:], mt1[:, :], u1[:, jo : jo + JW], start=False, stop=True
                )
                nc.vector.tensor_copy(G2[:, jo : jo + JW], pg[:, 0:JW])

            def ab_chunk(jc):
                # alternate G2 partition halves per chunk -> duo concurrency
                jo = jc * JW
                hb = 64 * (jc % 2)
                pa = abp.tile([128, JW], f32, tag="paA", name=f"paA{jc}")
                nc.tensor.matmul(
                    pa[:], cab[hb : hb + 64, 0:128],
                    G2[hb : hb + 64, jo : jo + JW], start=True, stop=True,
                )
                pb = abp.tile([32, JW], f32, tag="paB", name=f"paB{jc}")
                nc.tensor.matmul(
                    pb[:], cab[hb : hb + 64, 128:160],
                    G2[hb : hb + 64, jo : jo + JW], start=True, stop=True,
                )
                nc.scalar.activation(
                    W0[0:32, jo : jo + JW], pb[0:32, 0:JW], AF.Exp,
                    accum_out=sacc0[0:32, jc : jc + 1],
                )
                # alternate the 4-replica copy between ACT and DVE per chunk
                if jc % 2 == 0:
                    nc.scalar.copy(AsclR[:, jo : jo + JW], pa[:, 0:JW])
                else:
                    nc.vector.tensor_copy(AsclR[:, jo : jo + JW], pa[:, 0:JW])

            def chain_step(n, half):
                jo, w = half * (HW // 2), HW // 2
                dst = W0[n * 32 : (n + 1) * 32, jo : jo + w]
                src = W0[(n - 1) * 32 : n * 32, jo : jo + w]
                rep = AsclR[(n - 1) * 32 : n * 32, jo : jo + w]
                nc.vector.scalar_tensor_tensor(
                    dst, src, 1.0 / n, rep,
                    op0=ALU.mult, op1=ALU.mult,
                    accum_out=sacc0[n * 32 : (n + 1) * 32, half : half + 1],
                )

            def keep_warm(n, half):
                # Tiny matmul data-dependent on chain step (n, half) via a
                # 1-col DVE copy to a base-0 tile: spaces PE activity through
                # the chain so HAM stays at K=8/8.
                i = 2 * (n - 1) + half
                nc.vector.tensor_copy(
                    kwsrc[:, i : i + 1],
                    W0[n * 32 : n * 32 + 32, half * (HW // 2) : half * (HW // 2) + 1],
                )
                pz = abp.tile([32, JW], f32, tag="paB", name=f"kw{i}")
                nc.tensor.matmul(
                    pz[0:1, 0:64], kwsrc[:, i : i + 1], u0[0:32, 0:64],
                    start=True, stop=True,
                )

            # ---- vT matmuls (independent of chain; fills PE) ----
            def vt_tile(kt):
                pv = abp.tile([128, JW], f32, tag="paA", name=f"pv{kt}")
                ko = kt * 128
                nc.tensor.matmul(
                    pv[:, 0:C], u0[:, ko : ko + 128], vwt0[:],
                    start=True, stop=False,
                )
                nc.tensor.matmul(
                    pv[:, 0:C], u1[:, ko : ko + 128], vwt1[:],
                    start=False, stop=True,
                )
                nc.scalar.copy(vtb[:, kt * C : (kt + 1) * C], pv[:, 0:C])

            # ---- HAM warm-up: ~3.8us of scratch matmuls, zero-dependency,
            # so the PE reaches K=8/8 while input DMAs stream ----
            for i in range(9):
                pwm = bigp.tile([128, JW], f32, tag="big", name=f"warm{i}")
                nc.tensor.matmul(
                    pwm[:], scr[:, 0:128], scr[:, 0:512], start=True, stop=True
                )

            # Emission order drives the Tile scheduler's priorities.
            # vT tiles (keys = u cols 0..KH-1; host rolls u's j axis per core
            # so its key half leads, and un-rolls P afterward) interleave into
            # the front so the PE FIFO always has ready work.
            for jc in range(NJC):
                g2_chunk(jc)
                ab_chunk(jc)
                vt_tile(2 * jc)
                vt_tile(2 * jc + 1)
            # chain on DVE in half-width steps; keep-warm matmuls every
            # ~2.2us bridge the PE through the chain
            for n in range(1, NC):
                for half in range(2):
                    chain_step(n, half)
                    keep_warm(n, half)

            # ---- sigma -> s -> rinv ----
            nc.vector.reduce_sum(sig0[:], sacc0[:], axis=AX.X)
            nc.vector.tensor_copy(sigb0[:], sig0[:])
            for kt in range(NKT):
                nc.tensor.matmul(
                    s_ps[:, kt : kt + 1],
                    pwt[:, kt * 128 : (kt + 1) * 128], sigb0[:],
                    start=True, stop=True,
                )
            nc.vector.reciprocal(rinv[:], s_ps[:])

            # ---- pws scaling interleaved with the moment matmul ----
            for kt in range(NKT):
                nc.vector.tensor_scalar(
                    pws[:, kt * COLS : (kt + 1) * COLS],
                    pwsb[:, kt * COLS : (kt + 1) * COLS],
                    rinv[:, kt : kt + 1], None, op0=ALU.mult,
                )
                nc.tensor.matmul(
                    mo_ps0[:],
                    pws[:, kt * COLS : (kt + 1) * COLS],
                    vtb[:, kt * C : (kt + 1) * C],
                    start=(kt == 0), stop=(kt == NKT - 1),
                )
            nc.scalar.copy(mo0[:], mo_ps0[:])

            # ---- P = Mo^T @ W -> DRAM (1024-wide output groups) ----
            for ct in range(2):
                for jg in range(4):
                    jo = jg * 1024
                    ost = osp.tile([128, 1024], bf16, tag="ost", name=f"ost{ct}_{jg}")
                    for q in range(2):
                        sl = slice(q * 512, (q + 1) * 512)
                        js = jo + q * 512
                        pp = bigp.tile([128, JW], f32, tag="big",
                                       name=f"pp{ct}_{jg}_{q}")
                        nc.tensor.matmul(
                            pp[:],
                            mo0[:, ct * 128 : (ct + 1) * 128],
                            W0[:, js : js + 512],
                            start=True, stop=True,
                        )
                        if q == 0:
                            nc.scalar.copy(ost[:, sl], pp[:])
                        else:
                            nc.vector.tensor_copy(ost[:, sl], pp[:])
                    q_eng = nc.sync if jg % 2 == 0 else nc.gpsimd
                    q_eng.dma_start(
                        p_out[ct * 128 : (ct + 1) * 128, jo : jo + 1024],
                        ost[:, 0:1024],
                    )

    nc.compile()
    return nc


def _get_built():
    global _BUILT
    if _BUILT is None:
        _BUILT = _build()
    return _BUILT


def _regions(kw1f, beta, tmin, tmax):
    """Region edges (sorted breakpoints in range, capped at RP-1) and the
    per-region active-set midpoints."""
    bp = -beta / np.where(np.abs(kw1f) < 1e-30, 1e-30, kw1f)
    inr = np.sort(bp[(bp > tmin) & (bp < tmax)])
    while len(inr) > RP - 1:       # merge narrowest adjacent regions
        gaps = np.diff(np.concatenate([[tmin], inr, [tmax]]))
        i = int(np.argmin(gaps[:-1] + gaps[1:]))
        inr = np.delete(inr, i)
    full = np.concatenate([[tmin - 1.0], inr, [tmax + 1.0]])
    tmid = 0.5 * (full[:-1] + full[1:])
    return inr, tmid


def _host_prep(boundary_map, uncertainty_map, key_w1, bn_scale, bn_bias,
               bn_mean, bn_var, key_w2, query_w, value_w):
    import ml_dtypes

    bf = ml_dtypes.bfloat16
    b, c, h, w = uncertainty_map.shape
    H0 = boundary_map.shape[2]
    idx = (np.arange(h) * H0) // h
    bm = boundary_map[:, 0][:, idx][:, :, idx].reshape(b, h * w).astype(np.float64)

    inv = bn_scale.astype(np.float64) / np.sqrt(bn_var.astype(np.float64) + 1e-5)
    beta = bn_bias.astype(np.float64) - bn_mean.astype(np.float64) * inv
    kw1f = key_w1[:, 0].astype(np.float64) * inv
    m_t = np.ascontiguousarray((key_w2.T @ query_w).T).astype(np.float64)  # [256, 64]
    m_t2 = np.concatenate([m_t, m_t], axis=1).astype(bf)                   # [256, 128]
    vw_t = np.ascontiguousarray(value_w.T).astype(bf)                      # [256, 256]

    in_maps = []
    for core in range(8):
        bi, kh = core // 2, core % 2
        t_full = bm[bi]
        tau = np.abs(t_full).max()
        edges, tmid = _regions(kw1f, beta, t_full.min(), t_full.max())
        R = len(edges) + 1
        masks = (kw1f[None, :] * tmid[:, None] + beta[None, :]) > 0   # [R, 64]
        ca = (masks * kw1f[None, :]) * tau                            # [R, 64]
        cb = masks * beta[None, :]
        cabm = np.zeros((MID, 5 * RP), np.float64)
        for r in range(4):                    # 4 replicas of tau*CA^T
            cabm[:, r * RP : r * RP + R] = ca.T
        cabm[:, 4 * RP : 4 * RP + R] = cb.T
        cab2 = np.concatenate([cabm, cabm], axis=0).astype(bf)        # [128, 160]

        tk = t_full[kh * KH : (kh + 1) * KH]
        reg = np.searchsorted(edges, tk)                              # [2048]
        tp = np.empty((NC, KH), np.float64)
        tp[0] = 1.0
        for n in range(1, NC):
            tp[n] = tp[n - 1] * (tk / tau)
        pw = np.zeros((KH, COLS), np.float64)
        pw[np.arange(KH)[None, :].repeat(NC, 0).ravel(),
           (np.arange(NC)[:, None] * RP + reg[None, :]).ravel()] = tp.ravel()
        # device layout: [128, NKT*COLS] (k-tile t at cols t*COLS)
        pw_dev = pw.reshape(NKT, 128, COLS).transpose(1, 0, 2).reshape(128, NKT * COLS)

        u = uncertainty_map[bi].reshape(c, h * w)
        u = np.ascontiguousarray(np.roll(u, -kh * KH, axis=1)).astype(bf)
        in_maps.append({
            "u_in": u,
            "mt_in": m_t2,
            "cab_in": cab2,
            "vwt_in": vw_t,
            "pw_in": np.ascontiguousarray(pw_dev).astype(bf),
            "pwt_in": np.ascontiguousarray(pw.T).astype(bf),
        })
    return in_maps


def kernel(boundary_map, uncertainty_map, key_w1, bn_scale, bn_bias,
           bn_mean, bn_var, key_w2, query_w, value_w, gamma):
    global LAST_RESULTS
    from concourse.bass_utils import run_bass_kernel_spmd

    nc = _get_built()
    in_maps = _host_prep(
        np.asarray(boundary_map), np.asarray(uncertainty_map), np.asarray(key_w1),
        np.asarray(bn_scale), np.asarray(bn_bias), np.asarray(bn_mean),
        np.asarray(bn_var), np.asarray(key_w2), np.asarray(query_w),
        np.asarray(value_w),
    )
    kwargs = {}
    if TRACE:
        kwargs["trace"] = True
        if TRACE_CORES is not None:
            kwargs["trace_cores"] = TRACE_CORES
    res = run_bass_kernel_spmd(nc, in_maps, core_ids=list(range(8)), **kwargs)
    LAST_RESULTS = res

    b, c, h, w = uncertainty_map.shape
    g = np.float32(np.asarray(gamma).reshape(-1)[0])
    out = np.empty((b, c, h * w), np.float32)
    um = np.asarray(uncertainty_map)
    for bi in range(b):
        P = (res.results[2 * bi]["p_out"].astype(np.float32)
             + np.roll(res.results[2 * bi + 1]["p_out"].astype(np.float32),
                       KH, axis=1))
        out[bi] = g * P + um[bi].reshape(c, h * w)
    return out.reshape(b, c, h, w)
